# revision 5
# baseline (speedup 1.0000x reference)
"""Trainium2 Bass kernel for the controlled-unitary problem.

reference semantics (control=0, num_qubits=13, dim=8192):
    mask bit = 1 << 12, so columns/rows with that bit set are idx 4096..8191.
    out[:, c0] = state[:, c0]                       (control bit off: untouched)
    out[:, c1] = state[:, c1] @ target[c1, c1]      (controlled unitary)

Device work: complex [256,4096] @ [4096,4096] GEMM = 3 real GEMMs (Gauss).
Sharding: output columns of the GEMM split 8 ways (each core gets a
[4096, 512] slab of the target block; every weight byte moves once).

Per-core kernel (v3):
  Gauss variant with the plane-combines pushed to the host where possible:
      m1 = a_r  . (b_r + b_i)        (bs = b_r + b_i made on-chip, DVE)
      m2 = (a_r + a_i) . b_i         (aps host-precomputed)
      m3 = (a_i - a_r) . b_r         (ams host-precomputed)
      C_r = m1 - m2,  C_i = m1 + m3
  DMA in = 3 A planes (6MB) + 2 B planes (8MB) = 14MB, ~33us at the
  observed ~420GB/s aggregate, under the 41us fp16 PE floor.
  - PE warmup matmuls on a zeroed tile raise the p-state to full clock
    during the DMA head, so real matmuls run at 2.4GHz from the start.
  - Within each chunk PE order is m2, m3, m1 so the DVE bs add has slack.
  - Last chunk is ordered m-tile-major so m-tile 0's combine + output DMA
    overlap m-tile 1's matmuls; outputs are fp16 (halves out DMA).
"""

import numpy as np

BATCH = 256
DIM = 8192
HALF = 4096
N_CORES = 8
NSH = HALF // N_CORES  # 512 output columns per core
KT = HALF // 128  # 32 k-tiles
MT = BATCH // 128  # 2 m-tiles
CHUNKS = [1, 1, 2, 4, 8, 8, 8]  # k-tiles per DMA chunk (sums to KT)
CHMAX = max(CHUNKS)
WARM = 8  # p-state warmup matmuls before real work

DT_NAME = "float16"

_CACHE = {}


def _build():
    import concourse.mybir as mybir
    import concourse.tile as tile
    from concourse import bacc

    DT = mybir.dt.float16
    F32 = mybir.dt.float32

    nc = bacc.Bacc("TRN2", target_bir_lowering=False, debug=False,
                   num_devices=N_CORES)

    a_r = nc.dram_tensor("a_r", [128, KT, BATCH], DT, kind="ExternalInput")
    a_ps = nc.dram_tensor("a_ps", [128, KT, BATCH], DT, kind="ExternalInput")
    a_ms = nc.dram_tensor("a_ms", [128, KT, BATCH], DT, kind="ExternalInput")
    b_r = nc.dram_tensor("b_r", [128, KT, NSH], DT, kind="ExternalInput")
    b_i = nc.dram_tensor("b_i", [128, KT, NSH], DT, kind="ExternalInput")
    c_r = nc.dram_tensor("c_r", [BATCH, NSH], DT, kind="ExternalOutput")
    c_i = nc.dram_tensor("c_i", [BATCH, NSH], DT, kind="ExternalOutput")

    with tile.TileContext(nc) as tc:
        with (
            tc.tile_pool(name="ap", bufs=4) as ap_pool,
            tc.tile_pool(name="bp", bufs=4) as bp_pool,
            tc.tile_pool(name="op", bufs=2) as o_pool,
            tc.tile_pool(name="wp", bufs=1) as w_pool,
            tc.tile_pool(name="ps", bufs=1, space="PSUM") as ps_pool,
        ):
            ps = {}
            for m in range(MT):
                for comp in ("m1", "m2", "m3"):
                    ps[(m, comp)] = ps_pool.tile(
                        [128, NSH], F32, name=f"ps_{m}_{comp}"
                    )
            ps_warm = ps_pool.tile([128, NSH], F32, name="ps_warm")

            # p-state warmup: keep the PE busy on zeros while input DMA
            # streams in, so the clock is ramped when real matmuls start.
            wtile = w_pool.tile([128, NSH], DT, name="warm")
            nc.vector.memset(wtile[:], 0.0)
            for _ in range(WARM):
                nc.tensor.matmul(ps_warm[:], wtile[:, :128], wtile[:],
                                 start=True, stop=True)

            k0 = 0
            for ci, ch in enumerate(CHUNKS):
                nb = 3 if ch == CHMAX else 2
                ar_t = ap_pool.tile([128, ch, BATCH], DT, name=f"ar{ch}", bufs=nb)
                aps_t = ap_pool.tile([128, ch, BATCH], DT, name=f"aps{ch}", bufs=nb)
                ams_t = ap_pool.tile([128, ch, BATCH], DT, name=f"ams{ch}", bufs=nb)
                br_t = bp_pool.tile([128, ch, NSH], DT, name=f"br{ch}", bufs=nb)
                bi_t = bp_pool.tile([128, ch, NSH], DT, name=f"bi{ch}", bufs=nb)
                bs_t = bp_pool.tile([128, ch, NSH], DT, name=f"bs{ch}", bufs=nb)
                ksl = slice(k0, k0 + ch)
                # two HWDGE rings: B planes split across both; A planes
                # alternate to even out cumulative bytes (3A=1.5B units).
                nc.sync.dma_start(br_t[:], b_r[:, ksl, :])
                nc.scalar.dma_start(bi_t[:], b_i[:, ksl, :])
                if ci % 2 == 0:
                    nc.sync.dma_start(aps_t[:], a_ps[:, ksl, :])
                    nc.scalar.dma_start(ar_t[:], a_r[:, ksl, :])
                    nc.scalar.dma_start(ams_t[:], a_ms[:, ksl, :])
                else:
                    nc.sync.dma_start(ar_t[:], a_r[:, ksl, :])
                    nc.sync.dma_start(ams_t[:], a_ms[:, ksl, :])
                    nc.scalar.dma_start(aps_t[:], a_ps[:, ksl, :])
                # the only on-chip operand prep: bs = b_r + b_i
                nc.vector.tensor_tensor(bs_t[:], br_t[:], bi_t[:],
                                        mybir.AluOpType.add)
                last_chunk = k0 + ch == KT
                operands = {
                    "m1": (ar_t, bs_t),
                    "m2": (aps_t, bi_t),
                    "m3": (ams_t, br_t),
                }

                def issue(comp, m, kk):
                    lhs_t, rhs_t = operands[comp]
                    k = k0 + kk
                    msl = slice(m * 128, (m + 1) * 128)
                    nc.tensor.matmul(
                        ps[(m, comp)][:], lhs_t[:, kk, msl],
                        rhs_t[:, kk, :], start=(k == 0),
                        stop=(last_chunk and kk == ch - 1),
                    )

                if not last_chunk:
                    # product-major: m2/m3 need only DMA'd planes, m1 last
                    # so the DVE has slack to produce bs.
                    for comp in ("m2", "m3", "m1"):
                        for m in range(MT):
                            for kk in range(ch):
                                issue(comp, m, kk)
                else:
                    # m-tile-major so m-tile 0's combine + output DMA overlap
                    # m-tile 1's matmuls. Product order m1, m3, m2: only m1
                    # needs a PSUM->SBUF staging copy (t1, on the Act engine);
                    # the vector ops then read m3/m2 straight from PSUM:
                    #   out_i  = m3 + t1 =  C_i
                    #   out_rn = m2 - t1 = -C_r   (negated back on the host)
                    for m in range(MT):
                        msl = slice(m * 128, (m + 1) * 128)
                        out_rn = o_pool.tile([128, NSH], DT, name=f"out_r{m}")
                        out_i = o_pool.tile([128, NSH], DT, name=f"out_i{m}")
                        t1 = o_pool.tile([128, NSH], F32, name=f"t1_{m}")
                        for kk in range(ch):
                            issue("m1", m, kk)
                        nc.scalar.activation(
                            t1[:], ps[(m, "m1")][:],
                            mybir.ActivationFunctionType.Copy)
                        for kk in range(ch):
                            issue("m3", m, kk)
                        nc.vector.tensor_tensor(
                            out_i[:], ps[(m, "m3")][:], t1[:],
                            mybir.AluOpType.add)
                        nc.scalar.dma_start(c_i[msl, :], out_i[:])
                        for kk in range(ch):
                            issue("m2", m, kk)
                        nc.vector.tensor_tensor(
                            out_rn[:], ps[(m, "m2")][:], t1[:],
                            mybir.AluOpType.subtract)
                        nc.sync.dma_start(c_r[msl, :], out_rn[:])
                k0 += ch

    nc.compile()
    return nc


def _get_nc():
    if "nc" not in _CACHE:
        _CACHE["nc"] = _build()
    return _CACHE["nc"]


def _pack_kxm(mat_t):
    # mat_t: [4096, F] (k-major) -> [128, KT, F] with k = kt*128 + p
    f = mat_t.shape[1]
    return np.ascontiguousarray(
        mat_t.reshape(KT, 128, f).transpose(1, 0, 2).astype(np.float16)
    )


def run_device(A, B, dt_name=DT_NAME, trace=False):
    """A: [256, 4096] complex64, B: [4096, 4096] complex64.
    Returns C = A @ B as [256, 4096] complex64 plus the raw results."""
    from concourse import bass_utils

    nc = _get_nc()

    at = A.T  # [4096, 256]
    ar = np.ascontiguousarray(at.real).astype(np.float32)
    ai = np.ascontiguousarray(at.imag).astype(np.float32)
    a_r = _pack_kxm(ar)
    a_ps = _pack_kxm(ar + ai)
    a_ms = _pack_kxm(ai - ar)
    br_full = B.real
    bi_full = B.imag

    in_maps = []
    for c in range(N_CORES):
        csl = slice(c * NSH, (c + 1) * NSH)
        in_maps.append({
            "a_r": a_r,
            "a_ps": a_ps,
            "a_ms": a_ms,
            "b_r": _pack_kxm(np.ascontiguousarray(br_full[:, csl])),
            "b_i": _pack_kxm(np.ascontiguousarray(bi_full[:, csl])),
        })

    res = bass_utils.run_bass_kernel_spmd(
        nc, in_maps, core_ids=list(range(N_CORES)), trace=trace
    )

    out = np.empty((BATCH, HALF), dtype=np.complex64)
    for c in range(N_CORES):
        csl = slice(c * NSH, (c + 1) * NSH)
        # device returns c_r negated (m2 - m1); flip sign here for free
        out.real[:, csl] = -res.results[c]["c_r"].astype(np.float32)
        out.imag[:, csl] = res.results[c]["c_i"].astype(np.float32)
    return out, res


def kernel(state, target_matrix, control, num_qubits):
    state = np.asarray(state)
    target_matrix = np.asarray(target_matrix)
    control = int(control)
    num_qubits = int(num_qubits)
    dim = 1 << num_qubits

    assert state.shape == (BATCH, DIM) and dim == DIM, (
        "kernel hardcoded for [256, 8192]"
    )

    mask = 1 << (num_qubits - control - 1)
    idx = np.arange(dim)
    c1 = idx[(idx & mask) != 0]  # columns with control bit set

    if control == 0:
        A = state[:, HALF:]
        B = target_matrix[HALF:, HALF:]
    else:
        A = state[:, c1]
        B = target_matrix[np.ix_(c1, c1)]
    A = np.ascontiguousarray(A, dtype=np.complex64)
    B = np.ascontiguousarray(B, dtype=np.complex64)

    C, _ = run_device(A, B)

    out = state.astype(np.complex64, copy=True)
    out[:, c1] = C
    return out
